# revision 1
# baseline (speedup 1.0000x reference)
"""GAT (dense adjacency, 4-head, concat) + BatchNorm + ReLU on 8 TRN2 cores.

Math: h = x@W per head; scores s[n,m] = ei[n]+ej[m] (rank-1!);
att = softmax_m(mask(leaky(s))). Since exp(leaky(s))/exp(ei[n]) =
max(q1[m], q2[m]*w[n]) with q1=exp(ej), q2=exp(0.2*ej), w=exp(-0.8*ei),
and the exp(ei[n]) factor cancels in the softmax normalization, each
device only does 2 elementwise passes over its [8192, 1024] score block
(computed transposed so PE aggregates without any transposes; a ones
column in the rhs yields the softmax denominator for free).

Sharding: rows (target nodes) split across 8 cores; every core computes
h for all nodes from x (cheap) instead of an all-gather.
"""

import sys

sys.path.insert(0, "/opt/trn_rl_repo")

import numpy as np
import ml_dtypes

import concourse.bass as bass
import concourse.mybir as mybir
from concourse import tile
from concourse.bass_utils import run_bass_kernel_spmd

F32 = mybir.dt.float32
BF16 = mybir.dt.bfloat16
AF = mybir.ActivationFunctionType
OP = mybir.AluOpType

N, IN, OUT, H = 8192, 128, 64, 4
NCORES = 8
EPS = 1e-5

# Fraction of (m-tile, head) units on "Path C" (ACT relu + 1 DVE op) vs
# "Path A" (2 DVE ops); balances the VectorE and ScalarE engines.
PATH_C_NUM, PATH_C_DEN = 7, 10


def legalize_waits(nc, max_waits=1):
    """Walrus in this container encodes at most one inline sem-wait per
    engine instruction; hoist extras onto single-wait NoOps placed before."""
    nid = 0
    for f in nc.m.functions:
        for bb in f.blocks:
            new = []
            for inst in bb.instructions:
                si = inst.sync_info
                if si is not None and si.on_wait and len(si.on_wait) > max_waits:
                    waits = list(si.on_wait)
                    head, tail = waits[:-max_waits], waits[-max_waits:]
                    for w in head:
                        nid += 1
                        new.append(mybir.InstNoOp(
                            name=f"LGW-{nid}", ins=[], outs=[],
                            engine=inst.engine,
                            sync_info=mybir.SyncInfo(on_wait=[w], on_update=[]),
                            bass_nofuse=True,
                        ))
                    inst.sync_info = mybir.SyncInfo(
                        on_wait=tail, on_update=list(si.on_update)
                    )
                new.append(inst)
            bb.instructions = new
    return nc


def build_kernel(n_nodes=N, n_cores=NCORES, reps=1):
    """Build the per-core Bass program (SPMD: same program, per-core inputs)."""
    nblk = n_nodes // n_cores          # rows (target nodes) per core
    n_mt = n_nodes // 128              # m-tiles (source-node tiles of 128)
    jc = min(512, nblk)                # column-chunk width for PE moving dim
    n_jt = nblk // jc                  # column chunks of the n block
    WAUG = H * 66                      # per head: 64 h cols + ei + ej

    # all small inputs packed into one tensor -> ONE DMA on ONE HW queue, so
    # the first PE matmul carries a single semaphore wait (walrus limit)
    CW = n_nodes + nblk + WAUG + H + 128
    nc = bass.Bass()
    consts_d = nc.dram_tensor("consts", [IN, CW], F32, kind="ExternalInput")
    adjT_d = nc.dram_tensor("adjT", [n_nodes, nblk], BF16, kind="ExternalInput")
    out_d = nc.dram_tensor("out", [H, OUT + 1, nblk], F32, kind="ExternalOutput")

    with tile.TileContext(nc) as tc:
      for _rep in range(reps):
        with (
            tc.tile_pool(name="const", bufs=1) as cpool,
            tc.tile_pool(name="persist", bufs=1) as ppool,
            tc.tile_pool(name="stream", bufs=3) as spool,
            tc.tile_pool(name="score", bufs=4) as epool,
        ):
            # ---- load constants (one DMA) ----
            consts = cpool.tile([IN, CW], F32, tag="consts")
            nc.sync.dma_start(consts[:], consts_d[:])
            xT = consts[:, 0:n_nodes]
            xTown = consts[:, n_nodes:n_nodes + nblk]
            waug = consts[:, n_nodes + nblk:n_nodes + nblk + WAUG]
            wa = consts[:, n_nodes + nblk + WAUG:n_nodes + nblk + WAUG + H]
            ones_row = consts[0:1, CW - 128:CW]   # [1,128] of 1.0

            # ---- persistent per-head state ----
            h_aug, q1, q2, nq1, w_bc = [], [], [], [], []
            for hd in range(H):
                h_aug.append(ppool.tile([128, n_mt * 65], BF16, tag=f"haug{hd}", name=f"haug{hd}"))
                q1.append(ppool.tile([128, n_mt], F32, tag=f"q1_{hd}", name=f"q1_{hd}"))
                q2.append(ppool.tile([128, n_mt], F32, tag=f"q2_{hd}", name=f"q2_{hd}"))
                nq1.append(ppool.tile([128, n_mt], F32, tag=f"nq1_{hd}", name=f"nq1_{hd}"))
                w_bc.append(ppool.tile([128, nblk], BF16, tag=f"wbc{hd}", name=f"wbc{hd}"))
                # ones column of h_aug (col 64 of each 65-block) survives the
                # h copies below
                nc.gpsimd.memset(h_aug[hd][:], 1.0)

            # ---- phase A: h, ei, ej for all nodes; w for own rows ----
            # ei/ej fused into the h matmul: Waug cols per head = [W | W@a_i | W@a_j]
            pha = tc.tile_pool(name="psA", bufs=2, space="PSUM")
            psA_pool = pha.__enter__()
            phw = tc.tile_pool(name="psW", bufs=1, space="PSUM")
            psW_pool = phw.__enter__()
            for t in range(n_mt):
                psA = psA_pool.tile([128, WAUG], F32, tag="psA")
                nc.tensor.matmul(
                    psA[:], xT[:, t * 128:(t + 1) * 128], waug[:],
                    start=True, stop=True,
                )
                for hd in range(H):
                    c0 = hd * 66
                    nc.scalar.activation(
                        h_aug[hd][:, t * 65:t * 65 + 64], psA[:, c0:c0 + 64], AF.Copy
                    )
                    nc.scalar.activation(
                        q1[hd][:, t:t + 1], psA[:, c0 + 65:c0 + 66], AF.Exp
                    )
                    nc.scalar.activation(
                        q2[hd][:, t:t + 1], psA[:, c0 + 65:c0 + 66], AF.Exp, scale=0.2
                    )
            for hd in range(H):
                nc.vector.tensor_scalar_mul(nq1[hd][:], q1[hd][:], -1.0)
                # w[n] = exp(-0.8 * ei[n]) for own rows, in free-dim layout:
                # ei_row = wa_i[hd] @ xTown via PE (lhsT free dim = 1)
                eiT = psA_pool.tile([1, nblk], F32, tag="eiT")
                for j in range(n_jt):
                    nc.tensor.matmul(
                        eiT[:, j * jc:(j + 1) * jc],
                        wa[:, hd:hd + 1], xTown[:, j * jc:(j + 1) * jc],
                        start=True, stop=True,
                    )
                # broadcast ei row to all partitions via PE (ones ⊗ row),
                # then w = exp(-0.8*ei) on the PSUM->SBUF copy
                ei_row = spool.tile([1, nblk], F32, tag="eirow")
                nc.scalar.activation(ei_row[:], eiT[:], AF.Copy)
                psW = psW_pool.tile([128, nblk], F32, tag="psW")
                for j in range(n_jt):
                    nc.tensor.matmul(
                        psW[:, j * jc:(j + 1) * jc],
                        ones_row[:, :], ei_row[0:1, j * jc:(j + 1) * jc],
                        start=True, stop=True,
                    )
                nc.scalar.activation(w_bc[hd][:], psW[:], AF.Exp, scale=-0.8)

            phw.__exit__(None, None, None)
            pha.__exit__(None, None, None)
            # all-engine barrier: afterwards every engine's vector clock has
            # observed phase A, so each phase-B matmul needs <=1 sem wait
            tc.strict_bb_all_engine_barrier()

            # ---- phase B: masked attention + aggregation over m-tiles ----
            phb = tc.tile_pool(name="psB", bufs=1, space="PSUM")
            psB_pool = phb.__enter__()
            ps_out = [
                psB_pool.tile([OUT + 1, nblk], F32, tag=f"psB{hd}", name=f"psB{hd}") for hd in range(H)
            ]
            unit = 0
            for t in range(n_mt):
                adjt = spool.tile([128, nblk], BF16, tag="adjt")
                nc.sync.dma_start(adjt[:], adjT_d[t * 128:(t + 1) * 128, :])
                for hd in range(H):
                    q1s = q1[hd][:, t:t + 1]
                    q2s = q2[hd][:, t:t + 1]
                    E = epool.tile([128, nblk], BF16, tag="E")
                    if unit % PATH_C_DEN < PATH_C_NUM:
                        # Path C: r = relu(q2*w - q1) on ACT; E = (r+q1)*adj on DVE
                        r = epool.tile([128, nblk], BF16, tag="r")
                        nc.scalar.activation(
                            r[:], w_bc[hd][:], AF.Relu,
                            bias=nq1[hd][:, t:t + 1], scale=q2s,
                        )
                        nc.vector.scalar_tensor_tensor(
                            E[:], r[:], q1s, adjt[:], OP.add, OP.mult
                        )
                    else:
                        # Path A: a = max(q2*w, q1); E = a*adj (both on DVE)
                        a = epool.tile([128, nblk], BF16, tag="r")
                        nc.vector.tensor_scalar(
                            a[:], w_bc[hd][:], q2s, q1s, OP.mult, OP.max
                        )
                        nc.vector.tensor_tensor(E[:], a[:], adjt[:], OP.mult)
                    unit += 1
                    for j in range(n_jt):
                        nc.tensor.matmul(
                            ps_out[hd][:, j * jc:(j + 1) * jc],
                            h_aug[hd][:, t * 65:(t + 1) * 65],
                            E[:, j * jc:(j + 1) * jc],
                            start=(t == 0), stop=(t == n_mt - 1),
                        )

            # ---- phase C: emit [h + rowsum] rows; normalization on host ----
            for hd in range(H):
                o = spool.tile([OUT + 1, nblk], F32, tag="onorm")
                nc.scalar.activation(o[:], ps_out[hd][:], AF.Copy)
                nc.sync.dma_start(out_d[hd], o[:])
            phb.__exit__(None, None, None)

    return nc


_CACHE = {}


def _get_nc(n_nodes, n_cores):
    key = (n_nodes, n_cores)
    if key not in _CACHE:
        _CACHE[key] = legalize_waits(build_kernel(n_nodes, n_cores))
    return _CACHE[key]


def make_in_maps(x, adj, W, a_i, a_j, n_cores=NCORES):
    n_nodes = x.shape[0]
    nblk = n_nodes // n_cores
    xT = np.ascontiguousarray(x.T).astype(np.float32)
    adjT = np.ascontiguousarray(adj.T).astype(ml_dtypes.bfloat16)
    WAUGW = H * 66
    waug = np.zeros((IN, H, 66), np.float32)
    wa = np.zeros((IN, H), np.float32)
    for hd in range(H):
        waug[:, hd, 0:64] = W[hd]
        waug[:, hd, 64] = W[hd] @ a_i[hd]
        waug[:, hd, 65] = W[hd] @ a_j[hd]
        wa[:, hd] = W[hd] @ a_i[hd]
    waug = waug.reshape(IN, WAUGW)
    maps = []
    for c in range(n_cores):
        sl = slice(c * nblk, (c + 1) * nblk)
        ones = np.zeros((IN, 128), np.float32)
        ones[0, :] = 1.0
        consts = np.concatenate(
            [xT, xT[:, sl], waug, wa, ones], axis=1
        ).astype(np.float32)
        maps.append({
            "consts": np.ascontiguousarray(consts),
            "adjT": np.ascontiguousarray(adjT[:, sl]),
        })
    return maps


def postprocess(results, gamma, beta, n_cores=NCORES):
    """Per-core [H, 65, nblk] -> full [N, H*OUT] with softmax-norm + BN + ReLU."""
    blocks = []
    for c in range(n_cores):
        r = results[c]["out"]                      # [H, 65, nblk]
        o = r[:, :OUT, :] / r[:, OUT:OUT + 1, :]   # softmax normalize
        # [H, OUT, nblk] -> [nblk, H*OUT]
        blocks.append(np.transpose(o, (2, 0, 1)).reshape(-1, H * OUT))
    out = np.concatenate(blocks, axis=0).astype(np.float32)
    mean = out.mean(axis=0)
    var = out.var(axis=0)
    out = (out - mean) * (1.0 / np.sqrt(var + EPS)) * gamma + beta
    return np.maximum(out, 0.0).astype(np.float32)


def kernel(x, adj, W, a_i, a_j, gamma, beta):
    nc = _get_nc(N, NCORES)
    in_maps = make_in_maps(x, adj, W, a_i, a_j, NCORES)
    res = run_bass_kernel_spmd(nc, in_maps, list(range(NCORES)))
    return postprocess(res.results, np.asarray(gamma), np.asarray(beta), NCORES)



# revision 2
# speedup vs baseline: 35.8143x; 35.8143x over previous
"""GAT (dense adjacency, 4-head, concat) + BatchNorm + ReLU on 8 TRN2 cores.

Math per head: h = x@W; scores s[n,m] = leaky(ei[n]+ej[m]) masked by adj;
att = softmax_m. Using exp(leaky(s)) = max(exp(s), exp(0.2 s)) and dividing
the softmax numerator+denominator by exp(ei[n])*exp(0.2*ej[m]):

    E[m,n]   = q2[m] * max(t[m], w[n]) * adj[m,n]
    num[o,n] = sum_m h_aug[m,o] * E[m,n]          (PE accumulation)

with t = exp(0.8*ej), w = exp(-0.8*ei), q2 = exp(0.2*ej), h_aug = [h | 1]
(the aug column yields the softmax denominator for free).

Per-element work is split across engines, chosen per (m-tile, head) unit:
  D: M = max(w_bc, t[m]) * q2[m]  via DVE tensor_scalar (4x, two scalars)
  P: the same op on GPSIMD (own ALU, partially-shared SBUF port)
  S: q2*max(t,w) = q1 + q2*relu(w-t)  (q1 = exp(ej)): ACT emits
     r = relu(w - t[m]); the q1 part becomes an extra matmul q1h_s·adjT,
     so the unit costs ACT+PE instead of DVE/GPSIMD.
All units then share ONE DVE tensor_tensor (2x) that masks 4 tile-quarters
at once against a stride-0-replicated two-m-tile adj block.

Sharding: rows (target nodes) split across 8 cores; each core computes h
for all nodes from x. Heads run in two passes of 2 (PSUM: 2 heads x
[65,1024] f32 accumulators = 4 banks), adj streamed once per pass. Phase A
(h/ej/exps) is emitted interleaved with pass 0 so every engine's stream
pipelines group-by-group.
"""

import sys

sys.path.insert(0, "/opt/trn_rl_repo")

import numpy as np
import ml_dtypes

import concourse.bass as bass
import concourse.mybir as mybir
from concourse import tile
from concourse.bass_utils import run_bass_kernel_spmd

F32 = mybir.dt.float32
BF16 = mybir.dt.bfloat16
AF = mybir.ActivationFunctionType
OP = mybir.AluOpType

N, IN, OUT, H = 8192, 128, 64, 4
NCORES = 8
EPS = 1e-5
# adj is uploaded as {0, CMASK}; masking for "ACC" t-pairs happens inside
# the adj DMA itself via SWDGE accum_op=min (M < CMASK always, min(M,0)=0).
# tt-masked pairs fold 1/CMASK into the exp biases instead.
CMASK = 16384.0
LOG_CMASK = float(np.log(CMASK))

# Per-unit path schedules. Pass-0 heads (0,1) never use S: their ACT relu
# would queue behind phase-A work and gate the fused mask op. Pass-1 heads
# (2,3) lean on S since ACT is nearly idle by then.
PATTERN01 = "PD"
PATTERN23 = "SPSPD"
# Fraction of eligible t-pairs whose masking rides the adj DMA (accum-min).
# Disabled: in the cost model the SWDGE descriptor generation charged to the
# GPSIMD engine cancels the DVE tensor_tensor savings.
ACC_NUM, ACC_DEN = 0, 6


def path_of(t, hd):
    if hd < 2:
        return PATTERN01[(t * 2 + hd) % len(PATTERN01)]
    return PATTERN23[(t * 2 + hd) % len(PATTERN23)]


def legalize_waits(nc, max_waits=1):
    """Walrus in this container encodes at most one inline sem-wait per
    engine instruction; hoist extras onto single-wait NoOps placed before."""
    nid = 0
    for f in nc.m.functions:
        for bb in f.blocks:
            new = []
            for inst in bb.instructions:
                si = inst.sync_info
                if si is not None and si.on_wait and len(si.on_wait) > max_waits:
                    waits = list(si.on_wait)
                    head, tail = waits[:-max_waits], waits[-max_waits:]
                    for w in head:
                        nid += 1
                        new.append(mybir.InstNoOp(
                            name=f"LGW-{nid}", ins=[], outs=[],
                            engine=inst.engine,
                            sync_info=mybir.SyncInfo(on_wait=[w], on_update=[]),
                            bass_nofuse=True,
                        ))
                    inst.sync_info = mybir.SyncInfo(
                        on_wait=tail, on_update=list(si.on_update)
                    )
                new.append(inst)
            bb.instructions = new
    return nc


def build_kernel(n_nodes=N, n_cores=NCORES, reps=1):
    """Build the per-core Bass program (SPMD: same program, per-core inputs)."""
    nblk = n_nodes // n_cores          # rows (target nodes) per core
    n_mt = n_nodes // 128              # m-tiles (source-node tiles of 128)
    jc = min(512, nblk)                # column-chunk width for PE moving dim
    n_jt = nblk // jc                  # column chunks of the n block
    gsz = min(8, n_mt)                 # t-tiles per phase-A/ej group
    n_g = (n_mt + gsz - 1) // gsz
    tp = min(2, n_mt)                  # t-tiles per adj DMA / fused-mask block
    assert n_mt % tp == 0 and gsz % tp == 0
    HW_ = H * OUT                      # 256 h columns across heads
    CWS = nblk + HW_ + H + H * 128     # xTown | W-cols | waj | wai-bcast

    def path_at(t, hd):
        # the final t-tiles stay off GPSIMD/ACT so their (deep, slow) queues
        # never gate the kernel tail; the first ones stay off ACT (S), whose
        # queue at that point is still full of phase-A work
        if t >= n_mt - 2:
            return "D"
        if t < 2:
            return "P" if hd % 2 == 0 else "D"
        return path_of(t, hd)

    # S-unit positions per head
    s_t = {hd: [t for t in range(n_mt) if path_at(t, hd) == "S"]
           for hd in range(H)}
    s_idx = {hd: {t: i for i, t in enumerate(s_t[hd])} for hd in range(H)}
    s_cnt = {hd: max(1, len(s_t[hd])) for hd in range(H)}

    nc = bass.Bass()
    constsS_d = nc.dram_tensor("constsS", [IN, CWS], BF16, kind="ExternalInput")
    xT_d = nc.dram_tensor("xT", [IN, n_nodes], BF16, kind="ExternalInput")
    adjT_d = nc.dram_tensor("adjT", [n_nodes, nblk], BF16, kind="ExternalInput")
    out_d = nc.dram_tensor("out", [H, OUT + 1, nblk], F32, kind="ExternalOutput")

    with tile.TileContext(nc) as tc:
      for _rep in range(reps):
        with (
            tc.tile_pool(name="const", bufs=1) as cpool,
            tc.tile_pool(name="persist", bufs=1) as ppool,
            tc.tile_pool(name="stream", bufs=3) as spool,
            tc.tile_pool(name="adjp", bufs=5) as apool,
            tc.tile_pool(name="score", bufs=3) as epool,
        ):
            # ---- load constants (small consts, then xT in group chunks) ----
            constsS = cpool.tile([IN, CWS], BF16, tag="constsS")
            nc.sync.dma_start(constsS[:], constsS_d[:])
            xT = cpool.tile([IN, n_nodes], BF16, tag="xT")
            for g in range(n_g):
                # SWDGE: its own queue, so the SP queue is free for adjT
                c0, c1 = g * gsz * 128, min((g + 1) * gsz * 128, n_nodes)
                nc.gpsimd.dma_start(xT[:, c0:c1], xT_d[:, c0:c1])
            xTown = constsS[:, 0:nblk]
            waug = constsS[:, nblk:nblk + HW_ + H]      # [W-cols | waj]
            waibc = constsS[:, nblk + HW_ + H:nblk + HW_ + H + H * 128]

            # ---- persistent per-head state ----
            # h_aug_all holds [h | 1] 65-blocks for all (t, hd), t-major
            h_aug_all = ppool.tile([128, n_mt * H * 65], BF16,
                                   tag="haugall", name="haugall")
            nc.vector.memset(h_aug_all[:, 64:n_mt * H * 65:65], 1.0)

            def h_aug(t, hd):
                b = (t * H + hd) * 65
                return h_aug_all[:, b:b + 65]

            q2h_s, q1h_s, w_bc = [], [], []
            for hd in range(H):
                q2h_s.append(ppool.tile([128, s_cnt[hd] * 65], BF16,
                                        tag=f"q2hs{hd}", name=f"q2hs{hd}"))
                q1h_s.append(ppool.tile([128, s_cnt[hd] * 65], BF16,
                                        tag=f"q1hs{hd}", name=f"q1hs{hd}"))
                w_bc.append(ppool.tile([128, nblk], BF16,
                                       tag=f"wbc{hd}", name=f"wbc{hd}"))
            # per-(t,hd) scalars, t-major layout [128, n_mt*H]
            t_sc = ppool.tile([128, n_mt * H], F32, tag="t_sc", name="t_sc")
            q2_sc = ppool.tile([128, n_mt * H], F32, tag="q2_sc", name="q2_sc")
            tC_sc = ppool.tile([128, n_mt * H], F32, tag="tC_sc", name="tC_sc")
            q2C_sc = ppool.tile([128, n_mt * H], F32, tag="q2C_sc",
                                name="q2C_sc")
            ntC_sc = ppool.tile([128, n_mt * H], F32, tag="ntC_sc",
                                name="ntC_sc")

            def sc(tile_, t, hd):
                return tile_[:, t * H + hd:t * H + hd + 1]

            negLC = ppool.tile([128, 1], F32, tag="negLC", name="negLC")
            nc.vector.memset(negLC[:], -LOG_CMASK)

            # ---- PSUM accumulators for pass 0 (heads 0,1) allocated first so
            # their banks never alias the transient phase-A tiles ----
            phb0 = tc.tile_pool(name="psB0", bufs=1, space="PSUM")
            psB0 = phb0.__enter__()
            ps_out0 = [
                psB0.tile([OUT + 1, nblk], F32, tag=f"psB0_{hd}", name=f"psB0_{hd}")
                for hd in range(2)
            ]

            # ---- pre-phase: w_bc[hd] = exp(-0.8*ei) on all partitions.
            # ei broadcast over partitions in ONE matmul per chunk via the
            # host-packed outer product waibc[hd] = wai[hd] (x) ones[128] ----
            with tc.tile_pool(name="psPre", bufs=2, space="PSUM") as psPre:
                # pass-0 heads first: their w_bc gates the very first mask ops
                for hd in (0, 1, 2, 3):
                    for j in range(n_jt):
                        psW = psPre.tile([128, jc], F32, tag="psW")
                        nc.tensor.matmul(
                            psW[:], waibc[:, hd * 128:(hd + 1) * 128],
                            xTown[:, j * jc:(j + 1) * jc],
                            start=True, stop=True,
                        )
                        nc.scalar.activation(
                            w_bc[hd][:, j * jc:(j + 1) * jc], psW[:],
                            AF.Exp, scale=-0.8,
                        )

            # ---- phase A group g: h+ej matmuls, exps, h evacs ----
            PSA_HALF = 512                       # bank-aligned halves
            def emit_phase_a(g, psA_pool):
                t0 = g * gsz
                gl = min(gsz, n_mt - t0)
                for tl in range(0, gl, tp):
                    tA = t0 + tl
                    psA = psA_pool.tile([128, tp * PSA_HALF], F32, tag="psA")
                    for a in range(tp):
                        nc.tensor.matmul(
                            psA[:, a * PSA_HALF:a * PSA_HALF + HW_ + H],
                            xT[:, (tA + a) * 128:(tA + a + 1) * 128], waug[:],
                            start=True, stop=True,
                        )
                    # per-(t,hd) scalars: ej cols sit at half-offset HW_
                    ej_in = psA[:].rearrange("p (a c) -> p a c", a=tp)[
                        :, :, HW_:HW_ + H]
                    ssl = slice(tA * H, (tA + tp) * H)
                    nc.scalar.activation(t_sc[:, ssl], ej_in, AF.Exp, scale=0.8)
                    nc.scalar.activation(q2_sc[:, ssl], ej_in, AF.Exp, scale=0.2)
                    nc.scalar.activation(tC_sc[:, ssl], ej_in, AF.Exp,
                                         scale=0.8, bias=negLC[:])
                    nc.scalar.activation(q2C_sc[:, ssl], ej_in, AF.Exp,
                                         scale=0.2, bias=negLC[:])
                    nc.scalar.activation(ntC_sc[:, ssl], tC_sc[:, ssl],
                                         AF.Copy, scale=-1.0)
                    # plain h evac: all tp*H head-blocks in ONE strided op
                    esrc = psA[:].rearrange("p (a c) -> p a c", a=tp)[
                        :, :, 0:HW_].rearrange(
                        "p a (hd c) -> p a hd c", hd=H)[:, :, :, 0:OUT]
                    edst = h_aug_all[:, tA * H * 65:(tA + tp) * H * 65] \
                        .rearrange("p (a hd c) -> p a hd c",
                                   a=tp, hd=H)[:, :, :, 0:OUT]
                    nc.scalar.activation(edst, esrc, AF.Copy)
                    for hd in range(H):
                        for a in range(tp):
                            t = tA + a
                            if t in s_idx[hd]:
                                si = s_idx[hd][t]
                                nc.scalar.activation(
                                    q2h_s[hd][:, si * 65:si * 65 + 64],
                                    psA[:, a * PSA_HALF + hd * OUT:
                                         a * PSA_HALF + (hd + 1) * OUT],
                                    AF.Copy, scale=sc(q2_sc, t, hd),
                                )
                                nc.scalar.activation(
                                    q2h_s[hd][:, si * 65 + 64:si * 65 + 65],
                                    sc(q2_sc, t, hd), AF.Copy,
                                )
                                # q1h' = (t/C) * q2h: the q1 contribution
                                # against the {0,C}-valued adj rhs
                                nc.vector.tensor_scalar_mul(
                                    q1h_s[hd][:, si * 65:(si + 1) * 65],
                                    q2h_s[hd][:, si * 65:(si + 1) * 65],
                                    sc(tC_sc, t, hd),
                                )

            # ---- phase B block: one t-pair of one pass ----
            # ACC pairs: the per-quarter max op writes E4 directly and the
            # adj DMA applies the mask via SWDGE accum_op=min (one DMA per
            # head). tt pairs: quarters go to M4, one fused DVE
            # tensor_tensor masks all of them against the {0,C} adj tile
            # (1/C pre-folded into the quarters via the exp biases).
            def emit_pass_block(ps_out, heads, t0, acc):
                nh = len(heads)
                adj_src = adjT_d[t0 * 128:(t0 + tp) * 128, :].rearrange(
                    "(a p) n -> p a n", p=128)
                E4 = epool.tile([128, nh * tp * nblk], BF16, tag="E4")
                if acc:
                    for k, hd in enumerate(heads):
                        for tl in range(tp):
                            t = t0 + tl
                            q = (k * tp + tl) * nblk
                            p = path_at(t, hd)
                            eng = nc.gpsimd if p == "P" else nc.vector
                            eng.tensor_scalar(
                                E4[:, q:q + nblk], w_bc[hd][:],
                                sc(t_sc, t, hd), sc(q2_sc, t, hd),
                                OP.max, OP.mult,
                            )
                        nc.gpsimd.dma_start(
                            E4[:, k * tp * nblk:(k + 1) * tp * nblk]
                            .rearrange("p (a n) -> p a n", a=tp),
                            adj_src, accum_op=OP.min,
                        )
                    adjt = None
                else:
                    adjt = apool.tile([128, tp * nblk], BF16, tag="adjt")
                    nc.sync.dma_start(
                        adjt[:].rearrange("p (a n) -> p a n", a=tp), adj_src,
                    )
                    adj_rep = adjt[:].rearrange("p (a n) -> p a n", a=1) \
                                     .to_broadcast([128, nh, tp * nblk])
                    M4 = epool.tile([128, nh * tp * nblk], BF16, tag="M4")
                    for k, hd in enumerate(heads):
                        for tl in range(tp):
                            t = t0 + tl
                            q = (k * tp + tl) * nblk
                            p = path_at(t, hd)
                            if p == "S":
                                nc.scalar.activation(
                                    M4[:, q:q + nblk], w_bc[hd][:], AF.Relu,
                                    bias=sc(ntC_sc, t, hd), scale=1.0 / CMASK,
                                )
                            else:
                                eng = nc.gpsimd if p == "P" else nc.vector
                                eng.tensor_scalar(
                                    M4[:, q:q + nblk], w_bc[hd][:],
                                    sc(t_sc, t, hd), sc(q2C_sc, t, hd),
                                    OP.max, OP.mult,
                                )
                    nc.vector.tensor_tensor(E4[:], M4[:], adj_rep, OP.mult)
                for k, hd in enumerate(heads):
                    for tl in range(tp):
                        t = t0 + tl
                        q = (k * tp + tl) * nblk
                        is_s = path_at(t, hd) == "S"
                        lhsT = (q2h_s[hd][:, s_idx[hd][t] * 65:
                                          (s_idx[hd][t] + 1) * 65]
                                if is_s else h_aug(t, hd))
                        for j in range(n_jt):
                            nc.tensor.matmul(
                                ps_out[k][:, j * jc:(j + 1) * jc],
                                lhsT, E4[:, q + j * jc:q + (j + 1) * jc],
                                start=(t == 0), stop=(t == n_mt - 1 and not is_s),
                            )
                        if is_s:
                            si = s_idx[hd][t]
                            for j in range(n_jt):
                                nc.tensor.matmul(
                                    ps_out[k][:, j * jc:(j + 1) * jc],
                                    q1h_s[hd][:, si * 65:(si + 1) * 65],
                                    adjt[:, tl * nblk + j * jc:
                                          tl * nblk + (j + 1) * jc],
                                    start=False, stop=(t == n_mt - 1),
                                )

            def emit_pass_out(ps_out, heads):
                for k, hd in enumerate(heads):
                    o = spool.tile([OUT + 1, nblk], F32, tag="onorm")
                    nc.scalar.activation(o[:], ps_out[k][:], AF.Copy)
                    nc.sync.dma_start(out_d[hd], o[:])

            def acc_ok(heads, t0):
                # a pair is DMA-maskable if no quarter is S (S needs the raw
                # adj tile as a matmul rhs) and it is not in the head/tail
                # guard zone
                if t0 >= n_mt - 2:
                    return False
                return not any(path_at(t0 + tl, hd) == "S"
                               for tl in range(tp) for hd in heads)

            def want_acc(heads, t0, ctr):
                # convert ACC_NUM of every ACC_DEN eligible pairs
                return acc_ok(heads, t0) and (ctr % ACC_DEN) < ACC_NUM

            # ---- pass 0 (heads 0,1) interleaved with phase A, group-wise ----
            acc_ctr = 0
            with tc.tile_pool(name="psA", bufs=2, space="PSUM") as psA_pool:
                for g in range(n_g):
                    emit_phase_a(g, psA_pool)
                    for t0 in range(g * gsz, min((g + 1) * gsz, n_mt), tp):
                        acc = want_acc([0, 1], t0, acc_ctr)
                        acc_ctr += acc_ok([0, 1], t0)
                        emit_pass_block(ps_out0, [0, 1], t0, acc)
                emit_pass_out(ps_out0, [0, 1])
            phb0.__exit__(None, None, None)

            # ---- pass 1 (heads 2,3) ----
            with tc.tile_pool(name="psB1", bufs=1, space="PSUM") as psB1:
                ps_out1 = [
                    psB1.tile([OUT + 1, nblk], F32, tag=f"psB1_{hd}", name=f"psB1_{hd}")
                    for hd in range(2)
                ]
                for t0 in range(0, n_mt, tp):
                    acc = want_acc([2, 3], t0, acc_ctr)
                    acc_ctr += acc_ok([2, 3], t0)
                    emit_pass_block(ps_out1, [2, 3], t0, acc)
                emit_pass_out(ps_out1, [2, 3])

    return nc


_CACHE = {}


def _get_nc(n_nodes, n_cores):
    key = (n_nodes, n_cores)
    if key not in _CACHE:
        _CACHE[key] = legalize_waits(build_kernel(n_nodes, n_cores))
    return _CACHE[key]


def make_in_maps(x, adj, W, a_i, a_j, n_cores=NCORES):
    n_nodes = x.shape[0]
    nblk = n_nodes // n_cores
    xT = np.ascontiguousarray(np.asarray(x).T).astype(ml_dtypes.bfloat16)
    adjT = np.ascontiguousarray(
        np.asarray(adj).T.astype(np.float32) * CMASK).astype(ml_dtypes.bfloat16)
    HW_ = H * OUT
    wcols = np.zeros((IN, HW_), np.float32)
    waj = np.zeros((IN, H), np.float32)
    waibc = np.zeros((IN, H * 128), np.float32)
    # (cast to bf16 at pack time below)
    for hd in range(H):
        wcols[:, hd * OUT:(hd + 1) * OUT] = W[hd]
        waj[:, hd] = W[hd] @ a_j[hd]
        waibc[:, hd * 128:(hd + 1) * 128] = (W[hd] @ a_i[hd])[:, None]
    maps = []
    for c in range(n_cores):
        sl = slice(c * nblk, (c + 1) * nblk)
        constsS = np.concatenate(
            [xT[:, sl].astype(np.float32), wcols, waj, waibc], axis=1)
        maps.append({
            "constsS": np.ascontiguousarray(
                constsS.astype(ml_dtypes.bfloat16)),
            "xT": xT,
            "adjT": np.ascontiguousarray(adjT[:, sl]),
        })
    return maps


def postprocess(results, gamma, beta, n_cores=NCORES):
    """Per-core [H, 65, nblk] -> full [N, H*OUT] with softmax-norm + BN + ReLU."""
    blocks = []
    for c in range(n_cores):
        r = results[c]["out"]                      # [H, 65, nblk]
        o = r[:, :OUT, :] / r[:, OUT:OUT + 1, :]   # softmax normalize
        # [H, OUT, nblk] -> [nblk, H*OUT]
        blocks.append(np.transpose(o, (2, 0, 1)).reshape(-1, H * OUT))
    out = np.concatenate(blocks, axis=0).astype(np.float32)
    mean = out.mean(axis=0)
    var = out.var(axis=0)
    out = (out - mean) * (1.0 / np.sqrt(var + EPS)) * gamma + beta
    return np.maximum(out, 0.0).astype(np.float32)


def kernel(x, adj, W, a_i, a_j, gamma, beta):
    nc = _get_nc(N, NCORES)
    in_maps = make_in_maps(x, adj, W, a_i, a_j, NCORES)
    res = run_bass_kernel_spmd(nc, in_maps, list(range(NCORES)))
    return postprocess(res.results, np.asarray(gamma), np.asarray(beta), NCORES)


# revision 10
# speedup vs baseline: 36.1689x; 1.0099x over previous
"""GAT (dense adjacency, 4-head, concat) + BatchNorm + ReLU on 8 TRN2 cores.

Math per head: h = x@W; scores s[n,m] = leaky(ei[n]+ej[m]) masked by adj;
att = softmax_m. Using exp(leaky(s)) = max(exp(s), exp(0.2 s)) and dividing
the softmax numerator+denominator by exp(ei[n])*exp(0.2*ej[m]):

    E[m,n]   = q2[m] * max(t[m], w[n]) * adj[m,n]
    num[o,n] = sum_m h_aug[m,o] * E[m,n]          (PE accumulation)

with t = exp(0.8*ej), w = exp(-0.8*ei), q2 = exp(0.2*ej), h_aug = [h | 1]
(the aug column yields the softmax denominator for free).

Per-element work is split across engines, chosen per (m-tile, head) unit:
  D: M = max(w_bc, t[m]) * q2[m]  via DVE tensor_scalar (4x, two scalars)
  P: the same op on GPSIMD (own ALU, partially-shared SBUF port)
  S: q2*max(t,w) = q1 + q2*relu(w-t)  (q1 = exp(ej)): ACT emits
     r = relu(w - t[m]); the q1 part becomes an extra matmul q1h_s·adjT,
     so the unit costs ACT+PE instead of DVE/GPSIMD.
All units then share ONE DVE tensor_tensor (2x) that masks 4 tile-quarters
at once against a stride-0-replicated two-m-tile adj block.

Sharding: rows (target nodes) split across 8 cores; each core computes h
for all nodes from x. Heads run in two passes of 2 (PSUM: 2 heads x
[65,1024] f32 accumulators = 4 banks), adj streamed once per pass. Phase A
(h/ej/exps) is emitted interleaved with pass 0 so every engine's stream
pipelines group-by-group.
"""

import sys

sys.path.insert(0, "/opt/trn_rl_repo")

import numpy as np
import ml_dtypes

import concourse.bass as bass
import concourse.mybir as mybir
from concourse import tile
from concourse.bass_utils import run_bass_kernel_spmd

F32 = mybir.dt.float32
BF16 = mybir.dt.bfloat16
AF = mybir.ActivationFunctionType
OP = mybir.AluOpType

N, IN, OUT, H = 8192, 128, 64, 4
NCORES = 8
EPS = 1e-5
# adj is uploaded as {0, CMASK}; masking for "ACC" t-pairs happens inside
# the adj DMA itself via SWDGE accum_op=min (M < CMASK always, min(M,0)=0).
# tt-masked pairs fold 1/CMASK into the exp biases instead.
CMASK = 16384.0
LOG_CMASK = float(np.log(CMASK))

# Per-unit path schedules. Pass-0 heads (0,1) never use S: their ACT relu
# would queue behind phase-A work and gate the fused mask op. Pass-1 heads
# (2,3) lean on S since ACT is nearly idle by then.
PATTERN01 = "PD"
PATTERN23 = "SPSPD"
# late-half variants: bias ts work toward GPSIMD at the end of each pass so
# the DVE queue (which also owns every fused tensor_tensor) drains in time

# Fraction of eligible t-pairs whose masking rides the adj DMA (accum-min).
# Disabled: in the cost model the SWDGE descriptor generation charged to the
# GPSIMD engine cancels the DVE tensor_tensor savings.
ACC_NUM, ACC_DEN = 0, 6


def path_of(t, hd):
    if hd < 2:
        return PATTERN01[(t * 2 + hd) % len(PATTERN01)]
    return PATTERN23[(t * 2 + hd) % len(PATTERN23)]


def legalize_waits(nc, max_waits=1):
    """Walrus in this container encodes at most one inline sem-wait per
    engine instruction; hoist extras onto single-wait NoOps placed before."""
    nid = 0
    for f in nc.m.functions:
        for bb in f.blocks:
            new = []
            for inst in bb.instructions:
                si = inst.sync_info
                if si is not None and si.on_wait and len(si.on_wait) > max_waits:
                    waits = list(si.on_wait)
                    head, tail = waits[:-max_waits], waits[-max_waits:]
                    for w in head:
                        nid += 1
                        new.append(mybir.InstNoOp(
                            name=f"LGW-{nid}", ins=[], outs=[],
                            engine=inst.engine,
                            sync_info=mybir.SyncInfo(on_wait=[w], on_update=[]),
                            bass_nofuse=True,
                        ))
                    inst.sync_info = mybir.SyncInfo(
                        on_wait=tail, on_update=list(si.on_update)
                    )
                new.append(inst)
            bb.instructions = new
    return nc


def build_kernel(n_nodes=N, n_cores=NCORES, reps=1):
    """Build the per-core Bass program (SPMD: same program, per-core inputs)."""
    nblk = n_nodes // n_cores          # rows (target nodes) per core
    n_mt = n_nodes // 128              # m-tiles (source-node tiles of 128)
    jc = min(512, nblk)                # column-chunk width for PE moving dim
    n_jt = nblk // jc                  # column chunks of the n block
    gsz = min(8, n_mt)                 # t-tiles per phase-A/ej group
    n_g = (n_mt + gsz - 1) // gsz
    tp = min(2, n_mt)                  # t-tiles per adj DMA / fused-mask block
    assert n_mt % tp == 0 and gsz % tp == 0
    HW_ = H * OUT                      # 256 h columns across heads
    CWW = HW_ + H + H * 128            # W-cols | waj | wai-bcast

    def path_at(t, hd):
        # the final t-tiles stay off GPSIMD/ACT so their (deep, slow) queues
        # never gate the kernel tail; the first ones stay off ACT (S), whose
        # queue at that point is still full of phase-A work
        if t >= n_mt - 2:
            return "D"
        if t < 2:
            return "P" if hd % 2 == 0 else "D"
        if hd < 2 and t >= 40:
            # late pass-0: phase-A ACT load has tapered, so a 1/3 S share
            # here relieves DVE/GPSIMD without gating pairs on the ACT queue
            return "SPD"[(t * 2 + hd) % 3]
        return path_of(t, hd)

    # S-unit positions per head
    s_t = {hd: [t for t in range(n_mt) if path_at(t, hd) == "S"]
           for hd in range(H)}
    s_idx = {hd: {t: i for i, t in enumerate(s_t[hd])} for hd in range(H)}
    s_cnt = {hd: max(1, len(s_t[hd])) for hd in range(H)}

    nc = bass.Bass()
    constsW_d = nc.dram_tensor("constsW", [IN, CWW], BF16, kind="ExternalInput")
    xTown_d = nc.dram_tensor("xTown", [IN, nblk], BF16, kind="ExternalInput")
    xT_d = nc.dram_tensor("xT", [IN, n_nodes], BF16, kind="ExternalInput")
    adjT_d = nc.dram_tensor("adjT", [n_nodes, nblk], BF16, kind="ExternalInput")
    out_d = nc.dram_tensor("out", [H, OUT + 1, nblk], F32, kind="ExternalOutput")

    with tile.TileContext(nc) as tc:
      for _rep in range(reps):
        with (
            tc.tile_pool(name="const", bufs=1) as cpool,
            tc.tile_pool(name="persist", bufs=1) as ppool,
            tc.tile_pool(name="stream", bufs=3) as spool,
            tc.tile_pool(name="adjp", bufs=5) as apool,
            tc.tile_pool(name="score", bufs=3) as epool,
        ):
            # ---- load constants: the small weight block first (it gates
            # the first h matmul + pre-phase), own-rows block second, xT in
            # group chunks on the SWDGE queue ----
            constsW = cpool.tile([IN, CWW], BF16, tag="constsW")
            nc.sync.dma_start(constsW[:], constsW_d[:])
            xTownT = cpool.tile([IN, nblk], BF16, tag="xTownT")
            nc.sync.dma_start(xTownT[:], xTown_d[:])
            xT = cpool.tile([IN, n_nodes], BF16, tag="xT")
            for g in range(n_g):
                # SWDGE: its own queue, so the SP queue is free for adjT
                c0, c1 = g * gsz * 128, min((g + 1) * gsz * 128, n_nodes)
                nc.gpsimd.dma_start(xT[:, c0:c1], xT_d[:, c0:c1])
            xTown = xTownT[:, 0:nblk]
            waug = constsW[:, 0:HW_ + H]                # [W-cols | waj]
            waibc = constsW[:, HW_ + H:HW_ + H + H * 128]

            # ---- persistent per-head state ----
            # h_aug_all holds [h | 1] 65-blocks for all (t, hd), t-major
            h_aug_all = ppool.tile([128, n_mt * H * 65], BF16,
                                   tag="haugall", name="haugall")
            nc.vector.memset(h_aug_all[:, 64:n_mt * H * 65:65], 1.0)

            def h_aug(t, hd):
                b = (t * H + hd) * 65
                return h_aug_all[:, b:b + 65]

            q2h_s, q1h_s, w_bc = [], [], []
            for hd in range(H):
                q2h_s.append(ppool.tile([128, s_cnt[hd] * 65], BF16,
                                        tag=f"q2hs{hd}", name=f"q2hs{hd}"))
                q1h_s.append(ppool.tile([128, s_cnt[hd] * 65], BF16,
                                        tag=f"q1hs{hd}", name=f"q1hs{hd}"))
                w_bc.append(ppool.tile([128, nblk], BF16,
                                       tag=f"wbc{hd}", name=f"wbc{hd}"))
            # per-(t,hd) scalars, t-major layout [128, n_mt*H]
            t_sc = ppool.tile([128, n_mt * H], F32, tag="t_sc", name="t_sc")
            q2_sc = ppool.tile([128, n_mt * H], F32, tag="q2_sc", name="q2_sc")
            tC_sc = ppool.tile([128, n_mt * H], F32, tag="tC_sc", name="tC_sc")
            q2C_sc = ppool.tile([128, n_mt * H], F32, tag="q2C_sc",
                                name="q2C_sc")
            ntC_sc = ppool.tile([128, n_mt * H], F32, tag="ntC_sc",
                                name="ntC_sc")

            def sc(tile_, t, hd):
                return tile_[:, t * H + hd:t * H + hd + 1]

            negLC = ppool.tile([128, 1], F32, tag="negLC", name="negLC")
            nc.vector.memset(negLC[:], -LOG_CMASK)
            # dummy exp issued immediately: pulls the one-time ACT
            # exp-table load (~2.7us) under the input-DMA latency instead of
            # paying it on the first w_bc exp of the critical head chain
            warmup = ppool.tile([128, 1], F32, tag="warmup", name="warmup")
            nc.scalar.activation(warmup[:], negLC[:], AF.Exp)

            # ---- PSUM accumulators for pass 0 (heads 0,1) allocated first so
            # their banks never alias the transient phase-A tiles ----
            phb0 = tc.tile_pool(name="psB0", bufs=1, space="PSUM")
            psB0 = phb0.__enter__()
            ps_out0 = [
                psB0.tile([OUT + 1, nblk], F32, tag=f"psB0_{hd}", name=f"psB0_{hd}")
                for hd in range(2)
            ]

            # ---- pre-phase: w_bc[hd] = exp(-0.8*ei) on all partitions.
            # ei broadcast over partitions in ONE matmul per chunk via the
            # host-packed outer product waibc[hd] = wai[hd] (x) ones[128] ----
            def emit_pre_phase():
                with tc.tile_pool(name="psPre", bufs=2, space="PSUM") as psPre:
                    # pass-0 heads first: their w_bc gates the first mask ops
                    for hd in (0, 1, 2, 3):
                        for j in range(n_jt):
                            psW = psPre.tile([128, jc], F32, tag="psW")
                            nc.tensor.matmul(
                                psW[:], waibc[:, hd * 128:(hd + 1) * 128],
                                xTown[:, j * jc:(j + 1) * jc],
                                start=True, stop=True,
                            )
                            nc.scalar.activation(
                                w_bc[hd][:, j * jc:(j + 1) * jc], psW[:],
                                AF.Exp, scale=-0.8,
                            )

            # ---- phase A group g: h+ej matmuls, exps, h evacs ----
            PSA_HALF = 512                       # bank-aligned halves
            def emit_phase_a(g, psA_pool):
                t0 = g * gsz
                gl = min(gsz, n_mt - t0)
                for tl in range(0, gl, tp):
                    tA = t0 + tl
                    if tA == 0:
                        continue             # pair 0 emitted up front
                    emit_phase_a_pair(tA, psA_pool)

            def emit_phase_a_pair(tA, psA_pool):
                if True:
                    psA = psA_pool.tile([128, tp * PSA_HALF], F32, tag="psA")
                    for a in range(tp):
                        nc.tensor.matmul(
                            psA[:, a * PSA_HALF:a * PSA_HALF + HW_ + H],
                            xT[:, (tA + a) * 128:(tA + a + 1) * 128], waug[:],
                            start=True, stop=True,
                        )
                    # per-(t,hd) scalars: ej cols sit at half-offset HW_
                    ej_in = psA[:].rearrange("p (a c) -> p a c", a=tp)[
                        :, :, HW_:HW_ + H]
                    ssl = slice(tA * H, (tA + tp) * H)
                    nc.scalar.activation(t_sc[:, ssl], ej_in, AF.Exp, scale=0.8)
                    nc.scalar.activation(q2_sc[:, ssl], ej_in, AF.Exp, scale=0.2)
                    nc.scalar.activation(tC_sc[:, ssl], ej_in, AF.Exp,
                                         scale=0.8, bias=negLC[:])
                    nc.scalar.activation(q2C_sc[:, ssl], ej_in, AF.Exp,
                                         scale=0.2, bias=negLC[:])
                    nc.scalar.activation(ntC_sc[:, ssl], tC_sc[:, ssl],
                                         AF.Copy, scale=-1.0)
                    # plain h evac: all tp*H head-blocks in ONE strided op
                    esrc = psA[:].rearrange("p (a c) -> p a c", a=tp)[
                        :, :, 0:HW_].rearrange(
                        "p a (hd c) -> p a hd c", hd=H)[:, :, :, 0:OUT]
                    edst = h_aug_all[:, tA * H * 65:(tA + tp) * H * 65] \
                        .rearrange("p (a hd c) -> p a hd c",
                                   a=tp, hd=H)[:, :, :, 0:OUT]
                    nc.scalar.activation(edst, esrc, AF.Copy)
                    for hd in range(H):
                        for a in range(tp):
                            t = tA + a
                            if t in s_idx[hd]:
                                si = s_idx[hd][t]
                                nc.scalar.activation(
                                    q2h_s[hd][:, si * 65:si * 65 + 64],
                                    psA[:, a * PSA_HALF + hd * OUT:
                                         a * PSA_HALF + (hd + 1) * OUT],
                                    AF.Copy, scale=sc(q2_sc, t, hd),
                                )
                                nc.scalar.activation(
                                    q2h_s[hd][:, si * 65 + 64:si * 65 + 65],
                                    sc(q2_sc, t, hd), AF.Copy,
                                )
                                # q1h' = (t/C) * q2h: the q1 contribution
                                # against the {0,C}-valued adj rhs
                                nc.vector.tensor_scalar_mul(
                                    q1h_s[hd][:, si * 65:(si + 1) * 65],
                                    q2h_s[hd][:, si * 65:(si + 1) * 65],
                                    sc(tC_sc, t, hd),
                                )

            # ---- phase B block: one t-pair of one pass ----
            # ACC pairs: the per-quarter max op writes E4 directly and the
            # adj DMA applies the mask via SWDGE accum_op=min (one DMA per
            # head). tt pairs: quarters go to M4, one fused DVE
            # tensor_tensor masks all of them against the {0,C} adj tile
            # (1/C pre-folded into the quarters via the exp biases).
            def emit_pass_block(ps_out, heads, t0, acc):
                nh = len(heads)
                adj_src = adjT_d[t0 * 128:(t0 + tp) * 128, :].rearrange(
                    "(a p) n -> p a n", p=128)
                E4 = epool.tile([128, nh * tp * nblk], BF16, tag="E4")
                if acc:
                    for k, hd in enumerate(heads):
                        for tl in range(tp):
                            t = t0 + tl
                            q = (k * tp + tl) * nblk
                            p = path_at(t, hd)
                            eng = nc.gpsimd if p == "P" else nc.vector
                            eng.tensor_scalar(
                                E4[:, q:q + nblk], w_bc[hd][:],
                                sc(t_sc, t, hd), sc(q2_sc, t, hd),
                                OP.max, OP.mult,
                            )
                        nc.gpsimd.dma_start(
                            E4[:, k * tp * nblk:(k + 1) * tp * nblk]
                            .rearrange("p (a n) -> p a n", a=tp),
                            adj_src, accum_op=OP.min,
                        )
                    adjt = None
                else:
                    adjt = apool.tile([128, tp * nblk], BF16, tag="adjt")
                    nc.sync.dma_start(
                        adjt[:].rearrange("p (a n) -> p a n", a=tp), adj_src,
                    )
                    adj_rep = adjt[:].rearrange("p (a n) -> p a n", a=1) \
                                     .to_broadcast([128, nh, tp * nblk])
                    M4 = epool.tile([128, nh * tp * nblk], BF16, tag="M4")
                    for k, hd in enumerate(heads):
                        for tl in range(tp):
                            t = t0 + tl
                            q = (k * tp + tl) * nblk
                            p = path_at(t, hd)
                            if p == "S":
                                nc.scalar.activation(
                                    M4[:, q:q + nblk], w_bc[hd][:], AF.Relu,
                                    bias=sc(ntC_sc, t, hd), scale=1.0 / CMASK,
                                )
                            else:
                                eng = nc.gpsimd if p == "P" else nc.vector
                                eng.tensor_scalar(
                                    M4[:, q:q + nblk], w_bc[hd][:],
                                    sc(t_sc, t, hd), sc(q2C_sc, t, hd),
                                    OP.max, OP.mult,
                                )
                    nc.vector.tensor_tensor(E4[:], M4[:], adj_rep, OP.mult)
                for k, hd in enumerate(heads):
                    for tl in range(tp):
                        t = t0 + tl
                        q = (k * tp + tl) * nblk
                        is_s = path_at(t, hd) == "S"
                        lhsT = (q2h_s[hd][:, s_idx[hd][t] * 65:
                                          (s_idx[hd][t] + 1) * 65]
                                if is_s else h_aug(t, hd))
                        for j in range(n_jt):
                            nc.tensor.matmul(
                                ps_out[k][:, j * jc:(j + 1) * jc],
                                lhsT, E4[:, q + j * jc:q + (j + 1) * jc],
                                start=(t == 0), stop=(t == n_mt - 1 and not is_s),
                            )
                        if is_s:
                            si = s_idx[hd][t]
                            for j in range(n_jt):
                                nc.tensor.matmul(
                                    ps_out[k][:, j * jc:(j + 1) * jc],
                                    q1h_s[hd][:, si * 65:(si + 1) * 65],
                                    adjt[:, tl * nblk + j * jc:
                                          tl * nblk + (j + 1) * jc],
                                    start=False, stop=(t == n_mt - 1),
                                )

            def emit_pass_out(ps_out, heads):
                for k, hd in enumerate(heads):
                    o = spool.tile([OUT + 1, nblk], F32, tag="onorm")
                    nc.scalar.activation(o[:], ps_out[k][:], AF.Copy)
                    nc.sync.dma_start(out_d[hd], o[:])

            def acc_ok(heads, t0):
                # a pair is DMA-maskable if no quarter is S (S needs the raw
                # adj tile as a matmul rhs) and it is not in the head/tail
                # guard zone
                if t0 >= n_mt - 2:
                    return False
                return not any(path_at(t0 + tl, hd) == "S"
                               for tl in range(tp) for hd in heads)

            def want_acc(heads, t0, ctr):
                # convert ACC_NUM of every ACC_DEN eligible pairs
                return acc_ok(heads, t0) and (ctr % ACC_DEN) < ACC_NUM

            # ---- pass 0 (heads 0,1) interleaved with phase A, group-wise.
            # Pair-0's phase A goes first (its exps gate the first mask ops),
            # then the pre-phase, then the groups. ----
            with tc.tile_pool(name="psA0", bufs=1, space="PSUM") as psA0_pool:
                emit_phase_a_pair(0, psA0_pool)
            emit_pre_phase()
            acc_ctr = 0
            with tc.tile_pool(name="psA", bufs=2, space="PSUM") as psA_pool:
                for g in range(n_g):
                    emit_phase_a(g, psA_pool)
                    for t0 in range(g * gsz, min((g + 1) * gsz, n_mt), tp):
                        acc = want_acc([0, 1], t0, acc_ctr)
                        acc_ctr += acc_ok([0, 1], t0)
                        emit_pass_block(ps_out0, [0, 1], t0, acc)
                emit_pass_out(ps_out0, [0, 1])
            phb0.__exit__(None, None, None)

            # ---- pass 1 (heads 2,3) ----
            with tc.tile_pool(name="psB1", bufs=1, space="PSUM") as psB1:
                ps_out1 = [
                    psB1.tile([OUT + 1, nblk], F32, tag=f"psB1_{hd}", name=f"psB1_{hd}")
                    for hd in range(2)
                ]
                for t0 in range(0, n_mt, tp):
                    acc = want_acc([2, 3], t0, acc_ctr)
                    acc_ctr += acc_ok([2, 3], t0)
                    emit_pass_block(ps_out1, [2, 3], t0, acc)
                emit_pass_out(ps_out1, [2, 3])

    return nc


_CACHE = {}


def _get_nc(n_nodes, n_cores):
    key = (n_nodes, n_cores)
    if key not in _CACHE:
        _CACHE[key] = legalize_waits(build_kernel(n_nodes, n_cores))
    return _CACHE[key]


def make_in_maps(x, adj, W, a_i, a_j, n_cores=NCORES):
    n_nodes = x.shape[0]
    nblk = n_nodes // n_cores
    xT = np.ascontiguousarray(np.asarray(x).T).astype(ml_dtypes.bfloat16)
    adjT = np.ascontiguousarray(
        np.asarray(adj).T.astype(np.float32) * CMASK).astype(ml_dtypes.bfloat16)
    HW_ = H * OUT
    wcols = np.zeros((IN, HW_), np.float32)
    waj = np.zeros((IN, H), np.float32)
    waibc = np.zeros((IN, H * 128), np.float32)
    # (cast to bf16 at pack time below)
    for hd in range(H):
        wcols[:, hd * OUT:(hd + 1) * OUT] = W[hd]
        waj[:, hd] = W[hd] @ a_j[hd]
        waibc[:, hd * 128:(hd + 1) * 128] = (W[hd] @ a_i[hd])[:, None]
    constsW = np.concatenate([wcols, waj, waibc], axis=1) \
        .astype(ml_dtypes.bfloat16)
    maps = []
    for c in range(n_cores):
        sl = slice(c * nblk, (c + 1) * nblk)
        maps.append({
            "constsW": np.ascontiguousarray(constsW),
            "xTown": np.ascontiguousarray(xT[:, sl]),
            "xT": xT,
            "adjT": np.ascontiguousarray(adjT[:, sl]),
        })
    return maps


def postprocess(results, gamma, beta, n_cores=NCORES):
    """Per-core [H, 65, nblk] -> full [N, H*OUT] with softmax-norm + BN + ReLU."""
    blocks = []
    for c in range(n_cores):
        r = results[c]["out"]                      # [H, 65, nblk]
        o = r[:, :OUT, :] / r[:, OUT:OUT + 1, :]   # softmax normalize
        # [H, OUT, nblk] -> [nblk, H*OUT]
        blocks.append(np.transpose(o, (2, 0, 1)).reshape(-1, H * OUT))
    out = np.concatenate(blocks, axis=0).astype(np.float32)
    mean = out.mean(axis=0)
    var = out.var(axis=0)
    out = (out - mean) * (1.0 / np.sqrt(var + EPS)) * gamma + beta
    return np.maximum(out, 0.0).astype(np.float32)


def kernel(x, adj, W, a_i, a_j, gamma, beta):
    nc = _get_nc(N, NCORES)
    in_maps = make_in_maps(x, adj, W, a_i, a_j, NCORES)
    res = run_bass_kernel_spmd(nc, in_maps, list(range(NCORES)))
    return postprocess(res.results, np.asarray(gamma), np.asarray(beta), NCORES)
